# revision 1
# baseline (speedup 1.0000x reference)
"""Equivariant block-diagonal linear (128x0e+128x1o+64x2e+32x3o) on 8 trn2 cores.

Strategy:
  - Data-parallel: x [50000, 1056] row-sharded into 8x [6250, 1056].
  - Per irrep r, the op is out[n, w*d+j] = sum_u w_r[u,w] * x[n, u*d+i] delta_ij,
    i.e. a dense matmul with WD_r = kron(w_r, I_d)  [mul*d, mul*d], built on host.
  - Per 128-node subtile: PE-transpose irrep-aligned f-chunks of x ([n,f] -> [f,n]
    in PSUM), copy to SBUF, then matmul with xT as the *stationary* operand and
    WD rows as the *moving* operand: out[n, g] = sum_f xT[f, n] * WD[f, g].
    The output lands in natural [node, feature] layout -> contiguous DMA out.
"""

import os
from contextlib import ExitStack

import numpy as np

import concourse.bass as bass
import concourse.tile as tile
from concourse import bacc, mybir
from concourse.bass_utils import run_bass_kernel_spmd

N_NODES = 50000
DIM = 1056
N_CORES = 8
SHARD = N_NODES // N_CORES  # 6250
P = 128  # nodes per subtile
N_SUB = (SHARD + P - 1) // P  # 49 (last has 106 rows)
N_SUB_RUN = int(os.environ.get("KERNEL_NSUB", str(N_SUB)))

IRREPS = [(128, 0), (128, 1), (64, 2), (32, 3)]
# per-irrep feature offset and span (= mul * (2l+1))
G_OFF = [0, 128, 512, 832]
G_SPAN = [128, 384, 320, 224]
# f-chunks (irrep-aligned, each <=128 wide): (global_off, local_off, width)
F_CHUNKS = [
    [(0, 0, 128)],
    [(128, 0, 128), (256, 128, 128), (384, 256, 128)],
    [(512, 0, 128), (640, 128, 128), (768, 256, 64)],
    [(832, 0, 128), (960, 128, 96)],
]

USE_FP32R = os.environ.get("KERNEL_FP32R", "1") == "1"
# fp32r matmul runs 1 cyc/row only when moving free dim >= 256 -> pad g-spans
G_PAD = [256, 384, 320, 256] if USE_FP32R else list(G_SPAN)

_cache = {}


def _build():
    key = ("prog", USE_FP32R, N_SUB_RUN)
    if key in _cache:
        return _cache[key]
    f32 = mybir.dt.float32
    f32r = mybir.dt.float32r
    nc = bacc.Bacc(
        "TRN2", target_bir_lowering=False, debug=False, num_devices=N_CORES
    )
    x_d = nc.dram_tensor("x", [SHARD, DIM], f32, kind="ExternalInput")
    wd_d = [
        nc.dram_tensor(f"wd{r}", [G_SPAN[r], G_PAD[r]], f32, kind="ExternalInput")
        for r in range(4)
    ]
    out_d = nc.dram_tensor("out", [SHARD, DIM], f32, kind="ExternalOutput")
    ident_d = nc.inline_tensor(np.eye(P, dtype=np.float32), name="ident")

    with ExitStack() as ctx:
        tc = ctx.enter_context(tile.TileContext(nc))
        wpool = ctx.enter_context(tc.tile_pool(name="w", bufs=1))
        xpool = ctx.enter_context(tc.tile_pool(name="xin", bufs=8))
        opool = ctx.enter_context(tc.tile_pool(name="oout", bufs=8))
        xtpool = ctx.enter_context(tc.tile_pool(name="xt", bufs=12))
        ps_t = ctx.enter_context(tc.tile_pool(name="ps_t", bufs=3, space="PSUM"))
        ps_o = ctx.enter_context(tc.tile_pool(name="ps_o", bufs=4, space="PSUM"))

        ident = wpool.tile([P, P], f32, tag="ident")
        nc.sync.dma_start(ident[:], ident_d[:])
        wsb = {}
        wdt = f32r if USE_FP32R else f32
        for r in range(4):
            for j, (_, fl, fw) in enumerate(F_CHUNKS[r]):
                t = wpool.tile([fw, G_PAD[r]], wdt, tag=f"w{r}_{j}")
                if USE_FP32R:
                    stg = wpool.tile([fw, G_PAD[r]], f32, tag=f"wstg{r}_{j}")
                    nc.sync.dma_start(stg[:], wd_d[r][fl : fl + fw, :])
                    nc.vector.tensor_copy(t[:], stg[:])
                else:
                    nc.sync.dma_start(t[:], wd_d[r][fl : fl + fw, :])
                wsb[(r, j)] = t

        ci = 0
        for s in range(N_SUB_RUN):
            rows = min(P, SHARD - s * P)
            xt_in = xpool.tile([P, DIM], f32, tag="x")
            half = DIM // 2
            nc.sync.dma_start(xt_in[:rows, :half], x_d[s * P : s * P + rows, :half])
            nc.gpsimd.dma_start(
                xt_in[:rows, half:], x_d[s * P : s * P + rows, half:]
            )
            out_t = opool.tile([P, DIM], f32, tag="o")
            for r in range(4):
                po = ps_o.tile([P, G_PAD[r]], f32, tag="po")
                nchunks = len(F_CHUNKS[r])
                for j, (fg, fl, fw) in enumerate(F_CHUNKS[r]):
                    pt = ps_t.tile([P, P], f32, tag="pt")
                    nc.tensor.transpose(
                        pt[:fw, :rows],
                        xt_in[:rows, fg : fg + fw],
                        ident[:rows, :rows],
                    )
                    xt_sb = xtpool.tile([P, P], wdt, tag="xt")
                    if ci % 2 == 0:
                        nc.vector.tensor_copy(xt_sb[:fw, :rows], pt[:fw, :rows])
                    else:
                        nc.scalar.copy(xt_sb[:fw, :rows], pt[:fw, :rows])
                    ci += 1
                    lhs = xt_sb[:fw, :rows]
                    rhs = wsb[(r, j)][:, :]
                    nc.tensor.matmul(
                        po[:rows, :],
                        lhs,
                        rhs,
                        start=(j == 0),
                        stop=(j == nchunks - 1),
                    )
                if ci % 2 == 0:
                    nc.vector.tensor_copy(
                        out_t[:rows, G_OFF[r] : G_OFF[r] + G_SPAN[r]],
                        po[:rows, : G_SPAN[r]],
                    )
                else:
                    nc.scalar.copy(
                        out_t[:rows, G_OFF[r] : G_OFF[r] + G_SPAN[r]],
                        po[:rows, : G_SPAN[r]],
                    )
                ci += 1
            nc.sync.dma_start(
                out_d[s * P : s * P + rows, :half], out_t[:rows, :half]
            )
            nc.gpsimd.dma_start(
                out_d[s * P : s * P + rows, half:], out_t[:rows, half:]
            )

    nc.compile()
    _cache[key] = nc
    return nc



# ---------------------------------------------------------------------------
# HT mode: host-transposed layout. Device sees xT [1056, SHARD] and writes
# outT [1056, SHARD]. W blocks are stationary (LDW amortized over node
# groups); xT chunks stream as the moving operand with N=512.
# ---------------------------------------------------------------------------
NG = 512  # nodes per group
N_GRP = (SHARD + NG - 1) // NG  # 13 (last = 106)
NBLK = 4  # groups per W-residency block

# all 128-aligned-ish g-chunks (same 9 chunks as F_CHUNKS, flat)
CHUNKS9 = [(r, fg, fl, fw) for r in range(4) for (fg, fl, fw) in F_CHUNKS[r]]
# W blocks keyed (r, jf, jg): [f-chunk jf, g-chunk jg] of irrep r
W_BLOCKS = []
for r in range(4):
    for jf, (_, fl, fw) in enumerate(F_CHUNKS[r]):
        for jg, (_, gl, gw) in enumerate(F_CHUNKS[r]):
            W_BLOCKS.append((r, jf, jg, fl, fw, gl, gw))


def _build_ht():
    key = ("ht", USE_FP32R)
    if key in _cache:
        return _cache[key]
    f32 = mybir.dt.float32
    f32r = mybir.dt.float32r
    mmdt = f32r if USE_FP32R else f32
    nc = bacc.Bacc(
        "TRN2", target_bir_lowering=False, debug=False, num_devices=N_CORES
    )
    xt_d = nc.dram_tensor("xt", [DIM, SHARD], f32, kind="ExternalInput")
    wd_d = [
        nc.dram_tensor(f"wd{r}", [G_SPAN[r], G_SPAN[r]], f32, kind="ExternalInput")
        for r in range(4)
    ]
    out_d = nc.dram_tensor("outt", [DIM, SHARD], f32, kind="ExternalOutput")

    with ExitStack() as ctx:
        tc = ctx.enter_context(tile.TileContext(nc))
        wpool = ctx.enter_context(tc.tile_pool(name="w", bufs=1))
        xpool = ctx.enter_context(tc.tile_pool(name="xin", bufs=1))
        opool = ctx.enter_context(tc.tile_pool(name="oout", bufs=10))
        ps_o = ctx.enter_context(tc.tile_pool(name="ps_o", bufs=6, space="PSUM"))

        # resident W blocks (rounded to f32r via staging copy when needed)
        wsb = {}
        for bi, (r, jf, jg, fl, fw, gl, gw) in enumerate(W_BLOCKS):
            t = wpool.tile([fw, gw], mmdt, tag=f"wb{bi}")
            if USE_FP32R:
                stg = wpool.tile([fw, gw], f32, tag=f"wstg{bi}")
                nc.sync.dma_start(stg[:], wd_d[r][fl : fl + fw, gl : gl + gw])
                nc.vector.tensor_copy(t[:], stg[:])
            else:
                nc.sync.dma_start(t[:], wd_d[r][fl : fl + fw, gl : gl + gw])
            wsb[(r, jf, jg)] = t

        dma_engines = [nc.sync, nc.gpsimd, nc.scalar]
        for blk0 in range(0, N_GRP, NBLK):
            grps = list(range(blk0, min(blk0 + NBLK, N_GRP)))
            # load xT chunks for these groups
            xts = {}
            for gi, g in enumerate(grps):
                cols = min(NG, SHARD - g * NG)
                for c9, (r, fg, fl, fw) in enumerate(CHUNKS9):
                    xin = xpool.tile([P, NG], f32r if USE_FP32R else f32, tag=f"xt{gi}_{c9}")
                    eng = dma_engines[(gi + c9) % len(dma_engines)]
                    eng.dma_start(
                        xin[:fw, :cols],
                        xt_d[fg : fg + fw, g * NG : g * NG + cols].bitcast(xin.dtype)
                        if USE_FP32R
                        else xt_d[fg : fg + fw, g * NG : g * NG + cols],
                    )
                    xts[(gi, c9)] = xin
            # chunk index within irrep -> global chunk9 index
            base9 = [0, 1, 4, 7]
            cc = 0
            for r in range(4):
                nch = len(F_CHUNKS[r])
                for jg in range(nch):
                    _, gl, gw = F_CHUNKS[r][jg]
                    goff = G_OFF[r] + gl
                    pos = []
                    for _gi in range(len(grps)):
                        po = ps_o.tile([P, NG], f32, tag="po")
                        pos.append(po)
                    for jf in range(nch):
                        blk = wsb[(r, jf, jg)]
                        for gi, g in enumerate(grps):
                            cols = min(NG, SHARD - g * NG)
                            c9 = base9[r] + jf
                            nc.tensor.matmul(
                                pos[gi][:gw, :cols],
                                blk[:, :],
                                xts[(gi, c9)][: blk.shape[0], :cols],
                                start=(jf == 0),
                                stop=(jf == nch - 1),
                            )
                    for gi, g in enumerate(grps):
                        cols = min(NG, SHARD - g * NG)
                        ot = opool.tile([P, NG], f32, tag="ot")
                        cc += 1
                        if (cc + gi) % 2 == 0:
                            nc.vector.tensor_copy(
                                ot[:gw, :cols], pos[gi][:gw, :cols]
                            )
                        else:
                            nc.scalar.copy(ot[:gw, :cols], pos[gi][:gw, :cols])
                        eng = dma_engines[(gi + jg) % len(dma_engines)]
                        eng.dma_start(
                            out_d[goff : goff + gw, g * NG : g * NG + cols],
                            ot[:gw, :cols],
                        )

    nc.compile()
    _cache[key] = nc
    return nc


def _dense_weights(ws):
    out = []
    for r, (mul, l) in enumerate(IRREPS):
        d = 2 * l + 1
        wd = np.kron(np.asarray(ws[r], dtype=np.float32), np.eye(d, dtype=np.float32))
        if G_PAD[r] != wd.shape[1]:
            wd = np.pad(wd, ((0, 0), (0, G_PAD[r] - wd.shape[1])))
        out.append(np.ascontiguousarray(wd, dtype=np.float32))
    return out


last_result = None  # BassKernelResults of the most recent run (for profiling)


MODE = os.environ.get("KERNEL_MODE", "ht")


def kernel(x, w0, w1, w2, w3):
    global last_result
    x = np.asarray(x, dtype=np.float32)
    wds = _dense_weights([w0, w1, w2, w3])
    trace = os.environ.get("KERNEL_TRACE", "0") == "1"
    if MODE == "ht":
        nc = _build_ht()
        in_maps = []
        for c in range(N_CORES):
            m = {"xt": np.ascontiguousarray(x[c * SHARD : (c + 1) * SHARD].T)}
            for r in range(4):
                m[f"wd{r}"] = wds[r][:, : G_SPAN[r]]
            in_maps.append(m)
        last_result = run_bass_kernel_spmd(
            nc, in_maps, core_ids=list(range(N_CORES)), trace=trace
        )
        return np.ascontiguousarray(
            np.concatenate([r["outt"].T for r in last_result.results], axis=0)
        )
    nc = _build()
    x = np.ascontiguousarray(x)
    in_maps = []
    for c in range(N_CORES):
        m = {"x": x[c * SHARD : (c + 1) * SHARD]}
        for r in range(4):
            m[f"wd{r}"] = wds[r]
        in_maps.append(m)
    last_result = run_bass_kernel_spmd(
        nc, in_maps, core_ids=list(range(N_CORES)), trace=trace
    )
    return np.concatenate([r["out"] for r in last_result.results], axis=0)



# revision 2
# speedup vs baseline: 2.2826x; 2.2826x over previous
"""Equivariant block-diagonal linear (128x0e+128x1o+64x2e+32x3o) on 8 trn2 cores.

Strategy (mode "pk", default):
  - Data-parallel: x [50000, 1056] row-sharded into 8x [6250, 1056].
  - Host repacks each shard per irrep into a [128, cols] bf16 layout with the
    multiplicity axis on partitions and (node, m-component) on the free axis:
      r0: [128, n]          cols (n)
      r1: [128, 3n]         cols (n, i)
      r2: [128, 5n/2]       two m-columns stacked on partitions (64*2)
      r3: [128, 7n/4(+pad)] four m-columns stacked on partitions (32*4)
    The per-irrep weights become dense [128,128] stationary operands:
      W0 = w0, W1 = w1, W2 = diag(w2, w2), W3 = diag(w3, w3, w3, w3).
  - Device: out[:, c] = W_r^T @ xp[:, c] as plain 128x128 bf16 matmuls with the
    512-col moving operand, PSUM -> bf16 SBUF copy, contiguous DMA both ways.
    Zero wasted FLOPs, DMA is pure bf16 (26.4 MB/core round trip).
  - Host unpacks the bf16 result back to [50000, 1056] f32.

bf16 quantization of x and w gives rel err ~2e-3 (gate is 2e-2).
"""

import os
from contextlib import ExitStack

import ml_dtypes
import numpy as np

import concourse.bass as bass
import concourse.tile as tile
from concourse import bacc, mybir
from concourse.bass_utils import run_bass_kernel_spmd

N_NODES = 50000
DIM = 1056
N_CORES = 8
SHARD = N_NODES // N_CORES  # 6250
P = 128

BF16 = np.dtype(ml_dtypes.bfloat16)

IRREPS = [(128, 0), (128, 1), (64, 2), (32, 3)]
G_OFF = [0, 128, 512, 832]

# packed column widths per irrep (node-major free axis)
CW = [SHARD, 3 * SHARD, 5 * SHARD // 2, (7 * SHARD + 2) // 4]  # pad r3 by 2 src cols
COFF = [0, CW[0], CW[0] + CW[1], CW[0] + CW[1] + CW[2]]
CT = sum(CW)  # 51563

TILE = int(os.environ.get("KERNEL_TILE", "2048"))

_cache = {}


def _build_pk():
    key = ("pk", TILE)
    if key in _cache:
        return _cache[key]
    f32 = mybir.dt.float32
    bf16 = mybir.dt.bfloat16
    nc = bacc.Bacc(
        "TRN2", target_bir_lowering=False, debug=False, num_devices=N_CORES
    )
    xp_d = nc.dram_tensor("xp", [P, CT], bf16, kind="ExternalInput")
    wp_d = nc.dram_tensor("wp", [P, 512], bf16, kind="ExternalInput")
    op_d = nc.dram_tensor("op", [P, CT], bf16, kind="ExternalOutput")

    with ExitStack() as ctx:
        tc = ctx.enter_context(tile.TileContext(nc))
        wpool = ctx.enter_context(tc.tile_pool(name="w", bufs=1))
        xpool = ctx.enter_context(tc.tile_pool(name="xin", bufs=4))
        opool = ctx.enter_context(tc.tile_pool(name="oout", bufs=4))
        pspool = ctx.enter_context(tc.tile_pool(name="ps", bufs=8, space="PSUM"))

        wsb = wpool.tile([P, 512], bf16, tag="w")
        nc.sync.dma_start(wsb[:], wp_d[:])

        ci = 0
        for r in range(4):
            c0r, c1r = COFF[r], COFF[r] + CW[r]
            lhs = wsb[:, r * P : (r + 1) * P]
            for c0 in range(c0r, c1r, TILE):
                cw = min(TILE, c1r - c0)
                xin = xpool.tile([P, TILE], bf16, tag="x")
                nc.sync.dma_start(xin[:, :cw], xp_d[:, c0 : c0 + cw])
                ot = opool.tile([P, TILE], bf16, tag="o")
                for s0 in range(0, cw, 512):
                    ns = min(512, cw - s0)
                    ps = pspool.tile([P, 512], f32, tag="ps")
                    nc.tensor.matmul(
                        ps[:, :ns],
                        lhs,
                        xin[:, s0 : s0 + ns],
                        start=True,
                        stop=True,
                    )
                    if ci % 2 == 0:
                        nc.vector.tensor_copy(ot[:, s0 : s0 + ns], ps[:, :ns])
                    else:
                        nc.scalar.copy(ot[:, s0 : s0 + ns], ps[:, :ns])
                    ci += 1
                nc.scalar.dma_start(op_d[:, c0 : c0 + cw], ot[:, :cw])

    nc.compile()
    _cache[key] = nc
    return nc


def _pack_weights(w0, w1, w2, w3):
    wp = np.zeros((P, 512), dtype=np.float32)
    wp[:, 0:128] = np.asarray(w0, dtype=np.float32)
    wp[:, 128:256] = np.asarray(w1, dtype=np.float32)
    w2 = np.asarray(w2, dtype=np.float32)
    for p in range(2):
        wp[p * 64 : (p + 1) * 64, 256 + p * 64 : 256 + (p + 1) * 64] = w2
    w3 = np.asarray(w3, dtype=np.float32)
    for p in range(4):
        wp[p * 32 : (p + 1) * 32, 384 + p * 32 : 384 + (p + 1) * 32] = w3
    return wp.astype(BF16)


def _pack_x(x):
    """x [50000, 1056] f32 -> list of 8 per-core [128, CT] bf16 arrays."""
    n = N_NODES
    a0 = np.ascontiguousarray(x[:, 0:128].T)  # [128, n]
    a1 = np.ascontiguousarray(
        x[:, 128:512].reshape(n, 128, 3).transpose(1, 0, 2).reshape(128, 3 * n)
    )
    b2 = x[:, 512:832].reshape(n, 64, 5).transpose(1, 0, 2).reshape(64, 5 * n)
    a2 = np.ascontiguousarray(
        b2.reshape(64, 5 * n // 2, 2).transpose(2, 0, 1).reshape(128, 5 * n // 2)
    )
    b3 = x[:, 832:1056].reshape(n, 32, 7).transpose(1, 0, 2).reshape(32, 7 * n)
    out = []
    for c in range(N_CORES):
        xp = np.empty((P, CT), dtype=BF16)
        xp[:, COFF[0] : COFF[0] + CW[0]] = a0[:, c * SHARD : (c + 1) * SHARD]
        xp[:, COFF[1] : COFF[1] + CW[1]] = a1[:, c * 3 * SHARD : (c + 1) * 3 * SHARD]
        xp[:, COFF[2] : COFF[2] + CW[2]] = a2[
            :, c * (5 * SHARD // 2) : (c + 1) * (5 * SHARD // 2)
        ]
        b3c = b3[:, c * 7 * SHARD : (c + 1) * 7 * SHARD]
        b3p = np.zeros((32, 4 * CW[3]), dtype=np.float32)
        b3p[:, : 7 * SHARD] = b3c
        xp[:, COFF[3] :] = (
            b3p.reshape(32, CW[3], 4).transpose(2, 0, 1).reshape(128, CW[3])
        )
        out.append(xp)
    return out


def _unpack_out(ops):
    """list of 8 [128, CT] bf16 -> [50000, 1056] f32."""
    out = np.empty((N_NODES, DIM), dtype=np.float32)
    for c, op in enumerate(ops):
        op = np.asarray(op)
        sl = slice(c * SHARD, (c + 1) * SHARD)
        y0 = op[:, COFF[0] : COFF[0] + CW[0]].astype(np.float32)
        out[sl, 0:128] = y0.T
        y1 = op[:, COFF[1] : COFF[1] + CW[1]].astype(np.float32)
        out[sl, 128:512] = y1.reshape(128, SHARD, 3).transpose(1, 0, 2).reshape(
            SHARD, 384
        )
        y2 = op[:, COFF[2] : COFF[2] + CW[2]].astype(np.float32)
        c2 = y2.reshape(2, 64, CW[2]).transpose(1, 2, 0).reshape(64, 5 * SHARD)
        out[sl, 512:832] = c2.reshape(64, SHARD, 5).transpose(1, 0, 2).reshape(
            SHARD, 320
        )
        y3 = op[:, COFF[3] : COFF[3] + CW[3]].astype(np.float32)
        c3 = y3.reshape(4, 32, CW[3]).transpose(1, 2, 0).reshape(32, 4 * CW[3])[
            :, : 7 * SHARD
        ]
        out[sl, 832:1056] = c3.reshape(32, SHARD, 7).transpose(1, 0, 2).reshape(
            SHARD, 224
        )
    return out


last_result = None  # BassKernelResults of the most recent run (for profiling)

MODE = os.environ.get("KERNEL_MODE", "pk")


def kernel(x, w0, w1, w2, w3):
    global last_result
    x = np.asarray(x, dtype=np.float32)
    trace = os.environ.get("KERNEL_TRACE", "0") == "1"
    nc = _build_pk()
    wp = _pack_weights(w0, w1, w2, w3)
    xps = _pack_x(x)
    in_maps = [{"xp": xps[c], "wp": wp} for c in range(N_CORES)]
    last_result = run_bass_kernel_spmd(
        nc, in_maps, core_ids=list(range(N_CORES)), trace=trace
    )
    return _unpack_out([r["op"] for r in last_result.results])
